# revision 9
# baseline (speedup 1.0000x reference)
"""Trainium2 Bass kernel for the Powderworld BehaviorFluidFlow step (v2).

Contract: kernel(**inputs) takes the FULL unsharded inputs
  world         (16, 20, 512, 512) f32
  rand_movement (16, 1, 512, 512) f32
  rand_interact (16, 1, 512, 512) f32   (unused by the reference)
  rand_element  (16, 1, 512, 512) f32   (unused by the reference)
and returns the FULL (16, 20, 512, 512) f32 output.

Sharding: data-parallel over batch; core k processes batches [2k, 2k+1].
All roll-based neighbor access is along W (axis 3), which stays local.

Design (vs the v1 per-channel copy_predicated blends, which left the DVE at
92% busy and 446us/core in the cost model against a ~240us DMA floor):

* Whole-pixel moves: each pass's blend gives every pixel the same source
  column across all 20 channels, so 18 payload channels are stored as bf16
  and packed two-per-int32.  A pass blend is then 2 copy_predicated ops
  over 10 int32 pair-channels instead of 4+ ops over 18+ f32 channels,
  halving the DVE blend element count.  bf16 (not fp16: fp16's denormal
  range starts at 6e-5 and flushes this data's small values to zero,
  failing the relative-error gate) rounds payloads by <= 2^-9, 10x inside
  the 2e-2 gate; ids/flags are exact.
* Channels 1 (density) and 6 (momentum) feed exact threshold compares in
  the pass-2 masks, so they stay unpacked f32 in a separate 2-channel tile
  blended with the same masks (exact bit copies).
* The element-set membership E = id in {0,3,8,9,12,14,15} and the
  not-did-gravity flag NDG = (didg < 0.5) | (id > 13.5) are per-pixel
  functions of the world, and the blend is a per-pixel selection, so they
  commute: E/NDG are computed ONCE from the input and blended through both
  passes as the 10th bf16 pair (0/1 values, exact).  Pass 2 and the final
  momentum fixup then need no membership recomputation at all - the fixup
  predicate is a strided read of the blended E channel.
* Engine split: DVE does the copy_predicated blends + the few threshold
  compares; GPSIMD does the membership chain (int-legal ops only - note
  int tensor_scalar ADD is not legal on Pool, so the +127 runs in f32
  before the int conversion) and the and-chains; Scalar does pack/unpack
  dtype conversions, plain copies and halo maintenance.  All engines land
  near the ~240us/core HBM floor.
"""
import sys

if '/opt/trn_rl_repo' not in sys.path:
    sys.path.insert(0, '/opt/trn_rl_repo')

import numpy as np
import concourse.bacc as bacc
import concourse.mybir as mybir
import concourse.tile as tile
from concourse.bass_utils import run_bass_kernel_spmd

A = mybir.AluOpType
F32 = mybir.dt.float32
BF16 = mybir.dt.bfloat16
I32 = mybir.dt.int32
I8 = mybir.dt.int8

B, C, H, W = 16, 20, 512, 512
N_CORES = 8
BPC = B // N_CORES
P = 128

_nc_cache = {}


def build_kernel(bpc=BPC, c=C, h=H, w=W):
    key = (bpc, c, h, w)
    if key in _nc_cache:
        return _nc_cache[key]

    nc = bacc.Bacc("TRN2", target_bir_lowering=False, debug=False,
                   num_devices=N_CORES)
    world = nc.dram_tensor("world", [bpc, c, h, w], F32, kind="ExternalInput")
    rand = nc.dram_tensor("rand", [bpc, h, w], F32, kind="ExternalInput")
    out = nc.dram_tensor("out", [bpc, c, h, w], F32, kind="ExternalOutput")

    WH = w + 2          # haloed width; data in cols [1, w], halos at 0, w+1
    FW = 2 * WH         # bf16 view width of the packed tile
    n_ht = h // P
    MAIN = slice(1, w + 1)
    NP = 10             # int32 pair-channels (9 world pairs + E/NDG pair)

    # packed pair p holds (half0, half1):
    #   p in 0..8:  half0 = world ch L0[p], half1 = world ch 11+p
    #   p == 9:     half0 = E (membership), half1 = NDG
    # L0 = [0, 2, 3, 4, 5, 7, 8, 9, 10]  (id=pair0, grav=pair1, didg=pair6)
    # SA load slices: world [0:1]->0, [2:6]->1:5, [7:11]->5:9

    H0_MAIN = slice(2, 2 + 2 * w, 2)        # bf16 view: half0 main cols
    H0_L = slice(0, 2 * w, 2)               # half0, j-1 neighbor
    H0_R = slice(4, 4 + 2 * w, 2)           # half0, j+1 neighbor
    H1_MAIN = slice(3, 3 + 2 * w, 2)
    H1_L = slice(1, 1 + 2 * w, 2)
    H1_R = slice(5, 5 + 2 * w, 2)

    # membership set {empty, water, lava, gas, acid, agentK, agentL}
    # = ids {0, 3, 8, 9, 12, 14, 15} = bits of 54025; tested by building
    # 1<<id via the f32 exponent-field trick (exact integer arithmetic).
    MBITS = 54025

    iters = [(b, t) for b in range(bpc) for t in range(n_ht)]
    n = len(iters)
    st = [dict() for _ in range(n)]   # per-iteration tile refs

    with tile.TileContext(nc) as tc:
        with tc.tile_pool(name="sin", bufs=3) as sin, \
             tc.tile_pool(name="sout", bufs=2) as sout, \
             tc.tile_pool(name="pk", bufs=2) as pkp, \
             tc.tile_pool(name="pk2", bufs=1) as pk2p, \
             tc.tile_pool(name="mf", bufs=2) as mfp, \
             tc.tile_pool(name="mf2", bufs=2) as mf2p, \
             tc.tile_pool(name="mk", bufs=7) as mk, \
             tc.tile_pool(name="it", bufs=2) as itp, \
             tc.tile_pool(name="dbl", bufs=2) as dblp, \
             tc.tile_pool(name="amf", bufs=3) as amfp, \
             tc.tile_pool(name="am", bufs=3) as amp, \
             tc.tile_pool(name="rp", bufs=2) as rp:

            def loads(i):
                b, t = iters[i]
                hs = slice(t * P, (t + 1) * P)
                s = st[i]
                s['SA'] = sin.tile([P, 9, WH], F32, tag="sin", name=f"SA{i}")
                s['SB'] = sin.tile([P, 9, WH], F32, tag="sin", name=f"SB{i}")
                s['MF1'] = mfp.tile([P, 2, WH], F32, tag="mf", name=f"MF1_{i}")
                s['RAND'] = rp.tile([P, w], F32, tag="rand", name=f"RAND{i}")
                SA, SB, MF = s['SA'], s['SB'], s['MF1']
                # one DMA per channel: 2D partition-outer SBUF APs with a
                # sequential 256KB HBM run each (a multi-channel rearrange
                # jumps 1MB per 2KB chunk and page-thrashes DRAM; reordered
                # sliced APs hit 'illegal partition step')
                for k, ch in enumerate((0, 2, 3, 4, 5, 7, 8, 9, 10)):
                    nc.sync.dma_start(SA[:, k, MAIN], world[b, ch, hs, :])
                for k in range(9):
                    nc.sync.dma_start(SB[:, k, MAIN], world[b, 11 + k, hs, :])
                nc.sync.dma_start(MF[:, 0, MAIN], world[b, 1, hs, :])
                nc.sync.dma_start(MF[:, 1, MAIN], world[b, 6, hs, :])
                nc.sync.dma_start(s['RAND'][:], rand[b, hs, :])
                for T in (SA, SB, MF):
                    nc.scalar.copy(T[:, :, 0:1], T[:, :, w:w + 1])
                    nc.scalar.copy(T[:, :, w + 1:w + 2], T[:, :, 1:2])

            def pack(i):
                s = st[i]
                s['PK1'] = pkp.tile([P, NP, WH], I32, tag="pk", name=f"PK1_{i}")
                f = s['PK1'][:].bitcast(BF16)
                nc.scalar.copy(f[:, 0:9, 0:FW:2], s['SA'][:, :, :])
                nc.scalar.copy(f[:, 0:9, 1:FW:2], s['SB'][:, :, :])

            def mask_pass1(i):
                """Pass-1 masks from the dense staging tiles + E/NDG pack."""
                s = st[i]
                SA, MF, RAND = s['SA'], s['MF1'], s['RAND']
                PK1 = s['PK1']
                pkf = PK1[:].bitcast(BF16)
                ID = SA[:, 0, MAIN]
                FS = mk.tile([P, w], F32, tag="mk", name="FS")
                AIR = mk.tile([P, w], F32, tag="mk", name="AIR")
                E = mk.tile([P, w], F32, tag="mk", name="E")
                NDG = mk.tile([P, w], F32, tag="mk", name="NDG")
                GB = mk.tile([P, w], F32, tag="mk", name="GB")
                DN = mk.tile([P, w], F32, tag="mk", name="DN")
                DBL = dblp.tile([P, WH], F32, tag="dbl", name="DBL")
                AMf = amfp.tile([P, WH], F32, tag="amf", name=f"AMf1_{i}")
                AM = amp.tile([P, WH], I8, tag="am", name=f"AM1_{i}")
                IDF = itp.tile([P, w], F32, tag="it", name="IDF")
                IT = itp.tile([P, w], I32, tag="it", name="IT")
                VT = itp.tile([P, w], I32, tag="it", name="VT")

                # membership E: (id+127)*2^23 built in f32 (Pool int ADD and
                # int AND are not legal), converted to int32 = f32 bit
                # pattern of 2^id; reading that back as f32 and converting
                # to int gives 1<<id exactly.  Only the AND runs on DVE.
                nc.gpsimd.tensor_scalar(IDF[:], ID, 127.0, None, A.add)
                nc.gpsimd.tensor_scalar(IDF[:], IDF[:], 8388608.0, None,
                                        A.mult)
                nc.gpsimd.tensor_copy(IT[:], IDF[:])
                nc.gpsimd.tensor_copy(VT[:], IT[:].bitcast(F32))
                nc.vector.tensor_scalar(VT[:], VT[:], MBITS, None,
                                        A.bitwise_and)
                nc.gpsimd.tensor_scalar(E[:], VT[:], 0, None, A.is_gt)
                nc.gpsimd.tensor_scalar(AIR[:], ID, 13.5, None, A.is_gt)
                nc.gpsimd.tensor_scalar(NDG[:], SA[:, 6, MAIN], 0.5, None,
                                        A.is_lt)
                # logical_or is not legal on Pool; 0/1 inputs make add
                # equivalent (downstream only tests zero vs nonzero, and the
                # value 2.0 is exact in bf16 through the blends)
                nc.gpsimd.tensor_tensor(NDG[:], NDG[:], AIR[:], A.add)
                # pack E/NDG as pair 9 (0/1 exact in bf16) + its halos
                nc.scalar.copy(pkf[:, 9, H0_MAIN], E[:])
                nc.scalar.copy(pkf[:, 9, H1_MAIN], NDG[:])
                nc.gpsimd.tensor_copy(PK1[:, 9:10, 0:1], PK1[:, 9:10, w:w + 1])
                nc.gpsimd.tensor_copy(PK1[:, 9:10, w + 1:w + 2],
                                      PK1[:, 9:10, 1:2])

                nc.gpsimd.tensor_tensor(FS[:], RAND[:], MF[:, 1, MAIN], A.add)
                nc.gpsimd.tensor_tensor(GB[:], SA[:, 1, MAIN], SA[:, 1, 0:w],
                                        A.mult)
                nc.vector.tensor_tensor(DN[:], MF[:, 0, MAIN], MF[:, 0, 0:w],
                                        A.is_gt)
                nc.vector.scalar_tensor_tensor(FS[:], FS[:], 0.5, DN[:],
                                               A.is_gt, A.logical_and)
                nc.gpsimd.tensor_tensor(E[:], E[:], NDG[:], A.mult)
                nc.gpsimd.tensor_tensor(FS[:], FS[:], E[:], A.mult)
                nc.gpsimd.tensor_tensor(DBL[:, MAIN], FS[:], GB[:], A.mult)
                nc.scalar.copy(DBL[:, w + 1:w + 2], DBL[:, 1:2])
                nc.vector.scalar_tensor_tensor(AMf[:, MAIN], DBL[:, 2:w + 2],
                                               0.0, DBL[:, MAIN],
                                               A.is_equal, A.logical_and)
                nc.vector.tensor_copy(AMf[:, 0:1], AMf[:, w:w + 1])
                nc.vector.tensor_copy(AMf[:, w + 1:w + 2], AMf[:, 1:2])
                nc.scalar.copy(AM[:], AMf[:])
                s['A1f'], s['A1'] = AMf, AM

            def mask_pass2(i):
                """Pass-2 masks from the blended tiles (E/NDG pre-blended)."""
                s = st[i]
                pkf = s['PK2'][:].bitcast(BF16)
                MF, RAND = s['MF2'], s['RAND']
                FS = mk.tile([P, w], F32, tag="mk", name="FS")
                EN = mk.tile([P, w], F32, tag="mk", name="EN")
                GB = mk.tile([P, w], F32, tag="mk", name="GB")
                DN = mk.tile([P, w], F32, tag="mk", name="DN")
                DBL = dblp.tile([P, WH], F32, tag="dbl", name="DBL")
                AMf = amfp.tile([P, WH], F32, tag="amf", name=f"AMf2_{i}")
                AM = amp.tile([P, WH], I8, tag="am", name=f"AM2_{i}")

                nc.vector.tensor_tensor(FS[:], RAND[:], MF[:, 1, MAIN], A.add)
                nc.vector.scalar_tensor_tensor(FS[:], s['A1f'][:, 2:w + 2],
                                               2.0, FS[:], A.mult, A.add)
                nc.gpsimd.tensor_tensor(EN[:], pkf[:, 9, H0_MAIN],
                                        pkf[:, 9, H1_MAIN], A.mult)
                nc.gpsimd.tensor_tensor(GB[:], pkf[:, 1, H0_MAIN],
                                        pkf[:, 1, H0_R], A.mult)
                nc.vector.tensor_tensor(DN[:], MF[:, 0, MAIN],
                                        MF[:, 0, 2:w + 2], A.is_gt)
                nc.vector.scalar_tensor_tensor(FS[:], FS[:], 0.5, DN[:],
                                               A.is_le, A.logical_and)
                nc.gpsimd.tensor_tensor(FS[:], FS[:], EN[:], A.mult)
                nc.gpsimd.tensor_tensor(DBL[:, MAIN], FS[:], GB[:], A.mult)
                nc.scalar.copy(DBL[:, 0:1], DBL[:, w:w + 1])
                nc.vector.scalar_tensor_tensor(AMf[:, MAIN], DBL[:, 0:w],
                                               0.0, DBL[:, MAIN],
                                               A.is_equal, A.logical_and)
                nc.vector.tensor_copy(AMf[:, 0:1], AMf[:, w:w + 1])
                nc.vector.tensor_copy(AMf[:, w + 1:w + 2], AMf[:, 1:2])
                nc.scalar.copy(AM[:], AMf[:])
                s['A2f'], s['A2'] = AMf, AM

            def blend_copies(i, which):
                """Pre-emittable plain copies of a pass blend (the 'neither
                moves' bulk).  Emitted as early as dependencies allow so the
                copy_predicated ops never wait on them.  int32 moves run on
                Pool (bit-exact); the Activation engine is fp32-internal and
                would value-convert int32 payloads."""
                s = st[i]
                src, msrc = s[f'PK{which}'], s[f'MF{which}']
                if which == 1:
                    dst = pk2p.tile([P, NP, WH], I32, tag="pk2",
                                    name=f"PK2_{i}")
                    mdst = mf2p.tile([P, 2, WH], F32, tag="mf2",
                                     name=f"MF2_{i}")
                    s['PK2'], s['MF2'] = dst, mdst
                else:
                    dst, mdst = s['PK1'], s['MF1']   # pass-1 tiles dead; reuse
                    s['PK3'], s['MF3'] = dst, mdst
                nc.gpsimd.tensor_copy(dst[:, :, MAIN], src[:, :, MAIN])
                nc.scalar.copy(mdst[:, :, MAIN], msrc[:, :, MAIN])

            def blend_cps(i, which):
                """The 2+2 predicated copies of a pass + post-blend halos."""
                s = st[i]
                src, msrc = s[f'PK{which}'], s[f'MF{which}']
                AMi = s[f'A{which}']
                if which == 1:
                    dst, mdst = s['PK2'], s['MF2']
                    a_sl, b_sl = slice(0, w), slice(2, w + 2)
                    bm_sl = slice(2, w + 2)
                else:
                    dst, mdst = s['PK3'], s['MF3']
                    a_sl, b_sl = slice(2, w + 2), slice(0, w)
                    bm_sl = slice(0, w)
                am = AMi[:, MAIN].unsqueeze(1).broadcast_to((P, NP, w))
                bm = AMi[:, bm_sl].unsqueeze(1).broadcast_to((P, NP, w))
                am2 = AMi[:, MAIN].unsqueeze(1).broadcast_to((P, 2, w))
                bm2 = AMi[:, bm_sl].unsqueeze(1).broadcast_to((P, 2, w))
                nc.vector.copy_predicated(dst[:, :, MAIN], am, src[:, :, a_sl])
                nc.vector.copy_predicated(dst[:, :, MAIN], bm, src[:, :, b_sl])
                nc.vector.copy_predicated(mdst[:, :, MAIN], am2,
                                          msrc[:, :, a_sl])
                nc.vector.copy_predicated(mdst[:, :, MAIN], bm2,
                                          msrc[:, :, b_sl])
                if which == 1:
                    nc.gpsimd.tensor_copy(dst[:, :, 0:1], dst[:, :, w:w + 1])
                    nc.gpsimd.tensor_copy(dst[:, :, w + 1:w + 2],
                                          dst[:, :, 1:2])
                    nc.scalar.copy(mdst[:, :, 0:1], mdst[:, :, w:w + 1])
                    nc.scalar.copy(mdst[:, :, w + 1:w + 2], mdst[:, :, 1:2])

            def blend(i, which):
                blend_copies(i, which)
                blend_cps(i, which)

            def fixup(i):
                s = st[i]
                pkf = s['PK3'][:].bitcast(BF16)
                NF = mk.tile([P, w], F32, tag="mk", name="NF")
                FLI = amp.tile([P, w], I8, tag="am", name=f"FLI{i}")
                # nf = 2*b1 - 2*b2 (f32 masks, exact small integers)
                nc.gpsimd.tensor_tensor(NF[:], s['A1f'][:, 2:w + 2],
                                        s['A2f'][:, 0:w], A.subtract)
                nc.gpsimd.tensor_scalar(NF[:], NF[:], 2.0, None, A.mult)
                # is_fluid(final) = blended E channel (pair 9, half 0)
                nc.scalar.copy(FLI[:], pkf[:, 9, H0_MAIN])
                nc.vector.copy_predicated(s['MF3'][:, 1, MAIN], FLI[:], NF[:])

            def unpack_store(i):
                b, t = iters[i]
                hs = slice(t * P, (t + 1) * P)
                s = st[i]
                pkf = s['PK3'][:].bitcast(BF16)
                OA = sout.tile([P, 9, w], F32, tag="sout", name=f"OA{i}")
                OB = sout.tile([P, 9, w], F32, tag="sout", name=f"OB{i}")
                nc.scalar.copy(OA[:, :, :], pkf[:, 0:9, H0_MAIN])
                nc.scalar.copy(OB[:, :, :], pkf[:, 0:9, H1_MAIN])
                for k, ch in enumerate((0, 2, 3, 4, 5, 7, 8, 9, 10)):
                    nc.scalar.dma_start(out[b, ch, hs, :], OA[:, k, :])
                for k in range(9):
                    nc.scalar.dma_start(out[b, 11 + k, hs, :], OB[:, k, :])
                MF3 = s['MF3']
                nc.scalar.dma_start(out[b, 1, hs, :], MF3[:, 0, MAIN])
                nc.scalar.dma_start(out[b, 6, hs, :], MF3[:, 1, MAIN])

            # ---- software-pipelined emission -------------------------------
            # Plain copies are emitted before the mask chains that precede
            # their predicated copies, so the DVE never waits on them
            # (pk2/mf2 are double-buffered so the early copy never sits in
            # an in-order queue ahead of the reads that free its buffer).
            loads(0)
            pack(0)
            mask_pass1(0)
            if n > 1:
                loads(1)
            blend(0, 1)
            if n > 1:
                pack(1)
                mask_pass1(1)
            for i in range(n):
                blend_copies(i, 2)
                mask_pass2(i)
                if i + 1 < n:
                    # after mask_pass2(i): its Pool reads of PK2(i) precede
                    # this copy in the in-order Pool queue (pk2 is bufs=1)
                    blend_copies(i + 1, 1)
                blend_cps(i, 2)
                if i + 1 < n:
                    blend_cps(i + 1, 1)
                fixup(i)
                unpack_store(i)
                if i + 2 < n:
                    loads(i + 2)
                    pack(i + 2)
                    mask_pass1(i + 2)

    nc.compile()
    _nc_cache[key] = nc
    return nc


def kernel(world, rand_movement, rand_interact, rand_element):
    del rand_interact, rand_element
    nc = build_kernel()
    in_maps = []
    for k in range(N_CORES):
        bs = slice(k * BPC, (k + 1) * BPC)
        in_maps.append({
            "world": np.ascontiguousarray(world[bs]),
            "rand": np.ascontiguousarray(rand_movement[bs, 0]),
        })
    res = run_bass_kernel_spmd(nc, in_maps, list(range(N_CORES)))
    return np.concatenate([res.results[k]["out"] for k in range(N_CORES)], axis=0)
